# revision 19
# baseline (speedup 1.0000x reference)
"""Bipartite graph attention layer on 8 Trainium2 NeuronCores.

Sharding: data-parallel over (batch b, n_src half). Core c handles
b = c // 2, rows n0 = (c % 2) * 1024 .. +1024 of feat_src; params and
feat_dst[b] replicated per b-pair.

Math (per batch b, head h):
  h_src = feat_src @ W[h]; h_dst = feat_dst @ W[h]
  s[n] = tanh(h_src[n]) . w_src[h];  d[m] = tanh(h_dst[m]) . w_dst[h]
  E[m, n] = exp(leaky_relu(s[n] + d[m], 0.2))
  feat_out[n] = (sum_m E[m,n] h_dst[m]) / (sum_m E[m,n]) + b

Key identity used to avoid materializing logits:
  leaky(x) = 0.2 x + 0.8 relu(x)  =>
  E = exp(0.2 s) * exp(0.2 d) * max(exp(0.8 s) exp(0.8 d), 1)
The per-n factor exp(0.2 s) cancels in the softmax ratio, so the kernel
accumulates E' = E / exp(0.2 s), elementwise over [128 m, 1024 n] tiles:
  DVE/Pool chunks:  E' = max(u8[n] * v1[m], v2[m])   (tensor_scalar)
  ACT chunks:       E' = relu(u8[n] * v1[m] - v2[m]) (activation Relu,
                    scale/bias per partition) + a per-head base term
                    sum_{m in ACT chunks} v2[m] rhs[m,:] folded into the
                    PSUM accumulation via a ones-row matmul
with u8 = exp(0.8 s), v1 = exp(d), v2 = exp(0.2 d). sum_m comes from an
extra ones-column appended to the matmul rhs. All fat f32 matmuls are
issued as float32r (1 PE cycle/row at >=256 moving cols vs 4 for f32).
Attention accumulates into two [128, 4*66] PSUM banks per head so the
softmax reciprocal batches 4 columns per instruction, and the
normalize+bias epilogue is a single scalar_tensor_tensor per n-tile.
sigmoid(g) is computed as 0.5 tanh(0.5 g) + 0.5 and elu via
relu(y) + exp(min(y, 0)) - 1 (stt-fused); Copy/Exp/Relu/Tanh share one
ACT table set so no table reloads occur.
"""

import sys

sys.path.insert(0, "/opt/trn_rl_repo")

import numpy as np

B, N_SRC, N_DST, IN_DIM, OUT_DIM, H = 4, 2048, 2048, 256, 64, 4
N = N_SRC // 2        # n_src rows per core
M = N_DST             # dst rows per core
NT = N // 128         # 8 n-tiles per core
MC = M // 128         # 16 m-chunks
RW = 66               # rhs width: 64 h_dst cols + ones col + pad col

_CACHE = {}

# Ep chunk engine assignment per head: 'd' = DVE max-form,
# 'a' = ACT relu-form (+ base correction), 'p' = Pool max-form.
EP_ASSIGN = "aa" + "pp" + "dddddddddddd"
assert len(EP_ASSIGN) == MC


def _build_program(loop_k=None):
    import concourse.bass as bass
    import concourse.tile as tile
    from concourse import mybir
    from concourse.masks import make_identity

    f32 = mybir.dt.float32
    bf16 = mybir.dt.bfloat16
    AF = mybir.ActivationFunctionType
    OP = mybir.AluOpType

    nc = bass.Bass()
    fsrc_d = nc.declare_dram_parameter("fsrc", [N, IN_DIM], f32, isOutput=False)
    fdst_d = nc.declare_dram_parameter("fdst", [M, IN_DIM], f32, isOutput=False)
    W_d = nc.declare_dram_parameter("W", [H, IN_DIM, OUT_DIM], f32, isOutput=False)
    b_d = nc.declare_dram_parameter("bias", [OUT_DIM], f32, isOutput=False)
    wsrc_d = nc.declare_dram_parameter("wsrc", [H, OUT_DIM], f32, isOutput=False)
    wdst_d = nc.declare_dram_parameter("wdst", [H, OUT_DIM], f32, isOutput=False)
    Hw_d = nc.declare_dram_parameter("Hw", [IN_DIM, IN_DIM], f32, isOutput=False)
    Hb_d = nc.declare_dram_parameter("Hb", [IN_DIM], f32, isOutput=False)
    out_d = nc.declare_dram_parameter("out", [N, IN_DIM], f32, isOutput=True)

    with tile.TileContext(nc) as tc:
        if loop_k is None:
            _emit(nc, tc, bass, mybir, make_identity, f32, bf16, AF, OP,
                  fsrc_d, fdst_d, W_d, b_d, wsrc_d, wdst_d, Hw_d, Hb_d, out_d)
        else:
            with tc.For_i(0, loop_k):
                _emit(nc, tc, bass, mybir, make_identity, f32, bf16, AF, OP,
                      fsrc_d, fdst_d, W_d, b_d, wsrc_d, wdst_d, Hw_d, Hb_d,
                      out_d)

    _split_sync_waits(nc, mybir)
    return nc


def _emit(nc, tc, bass, mybir, make_identity, f32, bf16, AF, OP,
          fsrc_d, fdst_d, W_d, b_d, wsrc_d, wdst_d, Hw_d, Hb_d, out_d):
    from contextlib import ExitStack

    f32r = mybir.dt.float32r

    ctx = ExitStack()
    with ctx:
        const = ctx.enter_context(tc.tile_pool(name="const", bufs=1))
        head_p = ctx.enter_context(tc.tile_pool(name="head", bufs=2))
        ep_p = ctx.enter_context(tc.tile_pool(name="ep", bufs=4))
        fin_p = ctx.enter_context(tc.tile_pool(name="fin", bufs=2))
        ps_sm = ctx.enter_context(tc.tile_pool(name="ps_sm", bufs=2, space="PSUM"))
        ps_acc = ctx.enter_context(tc.tile_pool(name="ps_acc", bufs=2, space="PSUM"))

        # ---------------- loads (small params first; batched 4-tile DMAs;
        # dispatch on SP + Pool so the ACT sequencer stays free) ----------
        W_sb = const.tile([128, 2, H, OUT_DIM], f32)
        W_src_ap = W_d.rearrange("h (c p) o -> c p h o", p=128)
        for c in range(2):
            nc.sync.dma_start(W_sb[:, c, :, :], W_src_ap[c])
        Hw_sb = const.tile([128, 2, IN_DIM], f32)
        nc.scalar.dma_start(Hw_sb, Hw_d.rearrange("(r p) i -> p r i", p=128))
        Hb_row = const.tile([1, IN_DIM], f32)
        nc.sync.dma_start(Hb_row, Hb_d[None, :])
        b_full = const.tile([1, H, OUT_DIM], f32)
        for h in range(H):
            nc.scalar.dma_start(b_full[:, h, :], b_d[None, :])
        # wsrc_colT[64*hh + o, pair] = w_src[2*pair + hh, o]
        wsrc_colT = const.tile([128, 2], f32)
        nc.sync.dma_start(wsrc_colT,
                          wsrc_d.rearrange("(pair hh) o -> (hh o) pair", hh=2))
        wdst_rows = const.tile([1, H, OUT_DIM], f32)
        nc.scalar.dma_start(wdst_rows, wdst_d[None, :, :])

        fdst_sb = const.tile([128, MC, IN_DIM], f32)
        fdst_ap = fdst_d.rearrange("(g r p) i -> g p r i", p=128, r=4)
        for gq in range(4):
            (nc.sync if gq % 2 == 0 else nc.scalar).dma_start(
                fdst_sb[:, 4 * gq:4 * (gq + 1), :], fdst_ap[gq])
        fsrc_sb = const.tile([128, NT, IN_DIM], f32)
        fsrc_ap = fsrc_d.rearrange("(g r p) i -> g p r i", p=128, r=4)
        for gq in range(2):
            (nc.sync if gq % 2 == 0 else nc.scalar).dma_start(
                fsrc_sb[:, 4 * gq:4 * (gq + 1), :], fsrc_ap[gq])

        ident = const.tile([128, 128], f32)
        make_identity(nc, ident)
        # f32r copies of DMA-delivered matmul operands (early: h_dst and
        # h_src matmuls consume W_r)
        W_r = const.tile([128, 2, H, OUT_DIM], f32r)
        nc.vector.tensor_copy(W_r.rearrange("p c h o -> p (c h o)"),
                              W_sb.rearrange("p c h o -> p (c h o)"))
        Hb_row_r = const.tile([1, IN_DIM], f32r)
        nc.gpsimd.tensor_copy(Hb_row_r, Hb_row)
        ones_col = const.tile([1, 128], f32)
        nc.vector.memset(ones_col, 1.0)
        ones_col_r = const.tile([1, 128], f32r)
        nc.vector.tensor_copy(ones_col_r, ones_col)
        ones512_b = const.tile([1, 512], bf16)
        nc.vector.memset(ones512_b, 1.0)
        ones128 = const.tile([128, 128], f32)
        nc.vector.memset(ones128, 1.0)

        # psum -> sbuf copies alternate DVE / ACT (GPSIMD cannot read PSUM)
        copy_rr = [nc.vector.tensor_copy, nc.scalar.copy]

        def pe_transpose4(dsts, srcs, ceng):
            # batch up to 4 [128,128] transposes into one psum bank + 1 copy
            ps = ps_sm.tile([128, 512], f32, tag="sm")
            for k, src in enumerate(srcs):
                nc.tensor.transpose(ps[:, 128 * k:128 * (k + 1)], src, ident)
            for k, dst in enumerate(dsts):
                if len(dsts) == 1:
                    ceng(dst, ps[:, 0:128 * len(srcs)])
                else:
                    ceng(dst, ps[:, 128 * k:128 * (k + 1)])

        def bcast_row(dst, row_ap, width):
            # dst[128, width] sbuf <- row_ap[1, width] replicated to all rows
            ps = ps_sm.tile([128, 512], f32, tag="sm")
            nc.tensor.matmul(ps[:, 0:width], ones_col, row_ap,
                             start=True, stop=True)
            nc.vector.tensor_copy(dst, ps[:, 0:width])

        # ---------------- transposes / broadcasts ----------------
        # transposed operand tiles are float32r so the fat projection
        # matmuls run at 1 PE cycle/row; their producers (copies / ACT)
        # round to f32r as the BIR verifier requires.
        fdstT = [const.tile([128, M], f32r, tag=f"fdstT{c}", name=f"fdstT{c}")
                 for c in range(2)]
        for c in range(2):
            for t0 in range(0, MC, 4):
                pe_transpose4(
                    [fdstT[c][:, 128 * t0:128 * (t0 + 4)]],
                    [fdst_sb[:, t, 128 * c:128 * (c + 1)]
                     for t in range(t0, t0 + 4)],
                    copy_rr[(t0 // 4 + c) % 2])
        fsrcT = [const.tile([128, N], f32r, tag=f"fsrcT{c}", name=f"fsrcT{c}")
                 for c in range(2)]
        for c in range(2):
            for t0 in range(0, NT, 4):
                pe_transpose4(
                    [fsrcT[c][:, 128 * t0:128 * (t0 + 4)]],
                    [fsrc_sb[:, t, 128 * c:128 * (c + 1)]
                     for t in range(t0, t0 + 4)],
                    copy_rr[(t0 // 4 + c) % 2])
        HwT = [const.tile([128, IN_DIM], f32r, tag=f"HwT{c}", name=f"HwT{c}")
               for c in range(2)]
        for c in range(2):
            pe_transpose4(
                [HwT[c][:, 0:256]],
                [Hw_sb[:, t, 128 * c:128 * (c + 1)] for t in range(2)],
                copy_rr[c % 2])
        # replicated w_src: wsrc_rep[64*hh + o, pair, c] = w_src[2*pair+hh, o]
        wsrc_rep = const.tile([128, 2, 128], f32r)
        for pair in range(2):
            nc.vector.tensor_scalar(wsrc_rep[:, pair, :], ones128,
                                    wsrc_colT[:, pair:pair + 1], None, OP.mult)
        # w_dst broadcast rows (bf16 so the tw mul gets the 2x DVE mode)
        wdst_row = const.tile([128, H, OUT_DIM], bf16)
        bcast_row(wdst_row.rearrange("p h o -> p (h o)"),
                  wdst_rows.rearrange("p h o -> p (h o)"), H * OUT_DIM)
        b_full_b = const.tile([128, H * OUT_DIM], f32)
        bcast_row(b_full_b, b_full.rearrange("p h o -> p (h o)"), H * OUT_DIM)

        # ---------------- h_dst for all heads + a_dst ----------------
        rhs_all = const.tile([128, H, MC, RW], bf16)
        nc.vector.memset(rhs_all[:, :, :, 64:65], 1.0)
        nc.vector.memset(rhs_all[:, :, :, 65:66], 0.0)
        th_all = const.tile([128, H, MC, OUT_DIM], bf16)
        a_dst = const.tile([128, H, MC], f32)
        v1 = const.tile([128, H, MC], f32)
        v2 = const.tile([128, H, MC], f32)
        negv2 = const.tile([128, H, MC], f32)
        n_act = EP_ASSIGN.count("a")
        if n_act:
            v2b = const.tile([128, H, MC], bf16)

        for mp in range(MC // 2):
            hd = ps_sm.tile([128, 2, H * OUT_DIM], f32, tag="hd", bufs=2)
            for q in range(2):
                mc = 2 * mp + q
                for c in range(2):
                    nc.tensor.matmul(
                        hd[:, q, :],
                        fdstT[c][:, 128 * mc:128 * (mc + 1)],
                        W_r[:, c, :, :].rearrange("p h o -> p (h o)"),
                        start=(c == 0), stop=(c == 1))
            # tanh straight from PSUM (ACT); bf16 rhs copy on DVE
            hd4 = hd.rearrange("p q (h o) -> p q h o", h=H)
            nc.scalar.activation(
                th_all[:, :, 2 * mp:2 * mp + 2, :].rearrange(
                    "p h q o -> p q h o"),
                hd4, AF.Tanh)
            nc.vector.tensor_copy(
                rhs_all[:, :, 2 * mp:2 * mp + 2, 0:OUT_DIM].rearrange(
                    "p h q o -> p q h o"),
                hd4)

        # ---------------- heads: attention (h_dst^T stationary, Ep moving;
        # 32 wide matmuls per head instead of 128 Ldweights+Matmult pairs) --
        feat_pre = const.tile([128, NT, H * OUT_DIM], bf16)
        for pair in range(2):
            # h_srcT for head pair: psum [128 (2h, o), 512] x2 blocks
            th_srcT = head_p.tile([128, N], f32r, tag="thsrc")
            for nb in range(2):
                hs = ps_sm.tile([128, 512], f32, tag="sm")
                for c in range(2):
                    nc.tensor.matmul(
                        hs,
                        W_r[:, c, 2 * pair:2 * pair + 2, :].rearrange(
                            "p h o -> p (h o)"),
                        fsrcT[c][:, 512 * nb:512 * (nb + 1)],
                        start=(c == 0), stop=(c == 1))
                nc.scalar.activation(th_srcT[:, 512 * nb:512 * (nb + 1)], hs,
                                     AF.Tanh)
            for hh in range(2):
                h = 2 * pair + hh
                # just-in-time per-head a_dst -> v1/v2 (keeps each engine's
                # queue position close to this head's Ep production)
                tw = head_p.tile([128, MC, OUT_DIM], bf16, tag="tw")
                nc.vector.tensor_mul(
                    tw, th_all[:, h, :, :],
                    wdst_row[:, h:h + 1, :].broadcast_to([128, MC, OUT_DIM]))
                nc.vector.tensor_reduce(a_dst[:, h, :], tw,
                                        mybir.AxisListType.X, OP.add)
                nc.scalar.activation(v1[:, h, :], a_dst[:, h, :], AF.Exp)
                nc.scalar.activation(v2[:, h, :], a_dst[:, h, :], AF.Exp,
                                     scale=0.2)
                if n_act:
                    nc.vector.tensor_scalar(negv2[:, h, :], v2[:, h, :],
                                            -1.0, None, OP.mult)
                    nc.vector.tensor_copy(v2b[:, h, :], v2[:, h, :])
                u8 = head_p.tile([128, N], bf16, tag="u8")
                for nb in range(2):
                    sb = ps_sm.tile([128, 512], f32, tag="sm")
                    nc.tensor.matmul(
                        sb, wsrc_rep[64 * hh:64 * (hh + 1), pair, :],
                        th_srcT[64 * hh:64 * (hh + 1),
                                512 * nb:512 * (nb + 1)],
                        start=True, stop=True)
                    nc.scalar.activation(u8[:, 512 * nb:512 * (nb + 1)], sb,
                                         AF.Exp, scale=0.8)
                # per-head base term over ACT-assigned chunks
                if n_act:
                    bps = ps_sm.tile([128, 512], f32, tag="hd", bufs=2,
                                     name=f"bps{h}")
                    first = True
                    for mc in range(MC):
                        if EP_ASSIGN[mc] != "a":
                            continue
                        nc.tensor.matmul(
                            bps[0:1, 0:RW], v2b[:, h, mc:mc + 1],
                            rhs_all[:, h, mc, :],
                            start=first, stop=(mc == MC - 1 or
                                               EP_ASSIGN[mc + 1:].count("a") == 0))
                        first = False
                    base_row = head_p.tile([1, RW], bf16, tag="base")
                    nc.vector.tensor_copy(base_row, bps[0:1, 0:RW])
                # produce all 16 Ep tiles (slow engines emitted first),
                # then two accB sweeps consume each tile twice
                eps = []
                for mc in range(MC):
                    e = EP_ASSIGN[mc]
                    Ep = ep_p.tile([128, N], bf16, tag="Ep", bufs=MC + 2,
                                   name=f"Ep{h}_{mc}")
                    if e == "a":
                        nc.scalar.activation(Ep, u8, AF.Relu,
                                             bias=negv2[:, h, mc:mc + 1],
                                             scale=v1[:, h, mc:mc + 1])
                    else:
                        eng = nc.vector if e == "d" else nc.gpsimd
                        eng.tensor_scalar(Ep, u8, v1[:, h, mc:mc + 1],
                                          v2[:, h, mc:mc + 1], OP.mult, OP.max)
                    eps.append(Ep)
                # accB[66, 512]: rows = 64 h_dst cols + denom + pad, cols = n.
                # Stationary rhs_all chunk, moving Ep half-tile.
                for nb in range(2):
                    accB = ps_acc.tile([66, 512], f32, tag="accB", bufs=2,
                                       name=f"accB{h}_{nb}")
                    if n_act:
                        nc.tensor.matmul(accB, base_row[:, 0:RW][0:1, 0:66],
                                         ones512_b, start=True, stop=False)
                    for mc in range(MC):
                        nc.tensor.matmul(
                            accB, rhs_all[:, h, mc, :],
                            eps[mc][:, 512 * nb:512 * (nb + 1)],
                            start=(mc == 0 and not n_act),
                            stop=(mc == MC - 1))
                    # drain: psum -> sbuf, transpose back, batched
                    # reciprocal, fused normalize+bias epilogue
                    acc_sb = head_p.tile([66, 512], f32, tag="accsb")
                    (nc.vector.tensor_copy if nb == 0 else nc.scalar.copy)(
                        acc_sb, accB)
                    accT = ps_acc.tile([128, 512], f32, tag="accT", bufs=2,
                                       name=f"accT{h}_{nb}")
                    for k in range(4):
                        nc.tensor.transpose(accT[:, 66 * k:66 * k + 66],
                                            acc_sb[:, 128 * k:128 * (k + 1)],
                                            ident[0:66, 0:66])
                    accT4 = accT[:, 0:264].rearrange("p (a b) -> p a b", a=4)
                    rec = ep_p.tile([128, 4], f32, tag="rec", bufs=4)
                    nc.vector.reciprocal(
                        rec, accT4[:, :, 64:65].rearrange("p a b -> p (a b)"))
                    for k in range(4):
                        ns = 4 * nb + k
                        nc.vector.scalar_tensor_tensor(
                            feat_pre[:, ns, 64 * h:64 * (h + 1)],
                            accT4[:, k, 0:64], rec[:, k:k + 1],
                            b_full_b[:, 64 * h:64 * (h + 1)],
                            OP.mult, OP.add)

        # ---------------- gate matmuls (sigmoid folded into final) -------
        # emitted after attention: the PE work slots into head-drain gaps
        fsrc_b = const.tile([128, NT, IN_DIM], bf16)
        tg_all = const.tile([128, NT, IN_DIM], bf16)
        for t in range(NT):
            g = ps_sm.tile([128, 512], f32, tag="hd", bufs=2,
                           name=f"g{t}")[:, 0:IN_DIM]
            for c in range(2):
                nc.tensor.matmul(g, fsrcT[c][:, 128 * t:128 * (t + 1)],
                                 HwT[c], start=(c == 0), stop=False)
            nc.tensor.matmul(g, ones_col_r, Hb_row_r,
                             start=False, stop=True)
            # tg = tanh(0.5 g); sigma(g) = 0.5 tg + 0.5 is folded into the
            # final combine: out = 0.5 (tg + 1) (z-1 - fsrc) + fsrc
            nc.scalar.activation(tg_all[:, t, :], g, AF.Tanh, scale=0.5)
            (nc.vector if t % 2 else nc.gpsimd).tensor_copy(
                fsrc_b[:, t, :], fsrc_sb[:, t, :])

        # ---------------- elu + gate + combine ----------------
        # y = feat_pre (bias already added); z-1 = elu(y);
        # out = 0.5 (tg+1) (z-1-fsrc) + fsrc
        for t in range(NT):
            dve = t in (0, 1, 3, 4, 6, 7)
            v = nc.vector if dve else nc.gpsimd
            y = feat_pre[:, t, :]
            mn = fin_p.tile([128, IN_DIM], bf16, tag="mn")
            v.tensor_scalar(mn, y, 0.0, None, OP.min)
            e = fin_p.tile([128, IN_DIM], bf16, tag="e")
            nc.scalar.activation(e, mn, AF.Exp)
            d = fin_p.tile([128, IN_DIM], bf16, tag="d")
            q = fin_p.tile([128, IN_DIM], bf16, tag="q")
            if dve:
                # z-1 = (max(y,0) + e) - 1; d = z-1 - fsrc  (stt-fused)
                z1 = fin_p.tile([128, IN_DIM], bf16, tag="z1")
                v.scalar_tensor_tensor(z1, y, 0.0, e, OP.max, OP.add)
                v.scalar_tensor_tensor(d, z1, -1.0, fsrc_b[:, t, :],
                                       OP.add, OP.subtract)
                v.scalar_tensor_tensor(q, tg_all[:, t, :], 1.0, d,
                                       OP.add, OP.mult)
            else:
                # Pool has no scalar_tensor_tensor
                rr = fin_p.tile([128, IN_DIM], bf16, tag="rr")
                v.tensor_scalar(rr, y, 0.0, -1.0, OP.max, OP.add)
                z1 = fin_p.tile([128, IN_DIM], bf16, tag="z1")
                v.tensor_add(z1, rr, e)
                v.tensor_sub(d, z1, fsrc_b[:, t, :])
                tp = fin_p.tile([128, IN_DIM], bf16, tag="tp")
                v.tensor_scalar(tp, tg_all[:, t, :], 1.0, None, OP.add)
                v.tensor_mul(q, d, tp)
            o = fin_p.tile([128, IN_DIM], f32, tag="o")
            (nc.vector.scalar_tensor_tensor if dve
             else nc.vector.scalar_tensor_tensor)(
                o, q, 0.5, fsrc_sb[:, t, :], OP.mult, OP.add)
            (nc.sync if t % 2 == 0 else nc.scalar).dma_start(
                out_d[128 * t:128 * (t + 1), :], o)


def _split_sync_waits(nc, mybir, max_waits=1, drain_max_waits=0):
    """Walrus for cayman here accepts at most one sem-wait per
    instruction (and none on Drain): move overflow waits onto preceding
    same-engine NOPs."""
    n_split = 0
    for f in nc.m.functions:
        for bb in f.blocks:
            il = bb.instructions
            i = 0
            while i < len(il):
                ins = il[i]
                si = ins.sync_info
                limit = (drain_max_waits
                         if type(ins).__name__ == "InstDrain" else max_waits)
                if si is not None and len(si.on_wait) > limit:
                    waits = list(si.on_wait)
                    keep = waits[-limit:] if limit > 0 else []
                    overflow = waits[:len(waits) - limit]
                    chunks = [overflow[j:j + max_waits]
                              for j in range(0, len(overflow), max_waits)]
                    pos = i
                    for chunk in chunks:
                        nop = mybir.InstNoOp(
                            name=f"I-waitsplit-{n_split}",
                            engine=ins.engine,
                            sync_info=mybir.SyncInfo(on_wait=chunk, on_update=[]),
                        )
                        n_split += 1
                        il.insert(pos, nop)
                        pos += 1
                        i += 1
                    ins.sync_info = mybir.SyncInfo(
                        on_wait=keep, on_update=list(si.on_update))
                i += 1
    return n_split


def _get_runner():
    if "runner" in _CACHE:
        return _CACHE["runner"]
    import jax
    from jax.sharding import Mesh, PartitionSpec
    from jax.experimental.shard_map import shard_map
    import concourse.mybir as mybir
    from concourse.bass2jax import (_bass_exec_p, install_neuronx_cc_hook,
                                    partition_id_tensor)

    nc = _build_program()
    install_neuronx_cc_hook()
    n_cores = 8

    in_names, out_names, out_avals = [], [], []
    for alloc in nc.m.functions[0].allocations:
        if not isinstance(alloc, mybir.MemoryLocationSet):
            continue
        name = alloc.memorylocations[0].name
        if alloc.kind == "ExternalInput":
            if (nc.partition_id_tensor is not None
                    and name == nc.partition_id_tensor.name):
                continue
            in_names.append(name)
        elif alloc.kind == "ExternalOutput":
            out_names.append(name)
            out_avals.append(jax.core.ShapedArray(
                tuple(alloc.tensor_shape), mybir.dt.np(alloc.dtype)))
    n_params = len(in_names)
    in_names_all = list(in_names) + list(out_names)
    if nc.partition_id_tensor is not None:
        in_names_all.append(nc.partition_id_tensor.name)

    def _body(*args):
        operands = list(args)
        if nc.partition_id_tensor is not None:
            operands.append(partition_id_tensor())
        return tuple(_bass_exec_p.bind(
            *operands,
            out_avals=tuple(out_avals),
            in_names=tuple(in_names_all),
            out_names=tuple(out_names),
            lowering_input_output_aliases=(),
            sim_require_finite=True,
            sim_require_nnan=True,
            nc=nc,
        ))

    devices = jax.devices()[:n_cores]
    mesh = Mesh(np.asarray(devices), ("core",))
    n_outs = len(out_names)
    sharded = jax.jit(
        shard_map(_body, mesh=mesh,
                  in_specs=(PartitionSpec("core"),) * (n_params + n_outs),
                  out_specs=(PartitionSpec("core"),) * n_outs,
                  check_rep=False),
        keep_unused=True,
    )
    runner = (sharded, in_names, out_names, out_avals)
    _CACHE["runner"] = runner
    return runner


def _shard_inputs(feat_src, feat_dst, W, b, w_src, w_dst, H_w, H_b):
    per_core = []
    for c in range(8):
        bb, half = c // 2, c % 2
        per_core.append({
            "fsrc": np.ascontiguousarray(feat_src[bb, N * half:N * (half + 1)]),
            "fdst": np.ascontiguousarray(feat_dst[bb]),
            "W": W, "bias": b, "wsrc": w_src, "wdst": w_dst,
            "Hw": H_w, "Hb": H_b,
        })
    return per_core


def kernel(feat_src, feat_dst, W, b, w_src, w_dst, H_w, H_b):
    feat_src = np.asarray(feat_src, np.float32)
    feat_dst = np.asarray(feat_dst, np.float32)
    args = [np.asarray(a, np.float32) for a in (W, b, w_src, w_dst, H_w, H_b)]
    sharded, in_names, out_names, out_avals = _get_runner()
    per_core = _shard_inputs(feat_src, feat_dst, *args)
    concat_in = [np.concatenate([per_core[c][nm] for c in range(8)], axis=0)
                 for nm in in_names]
    concat_zeros = [np.zeros((8 * av.shape[0], *av.shape[1:]), av.dtype)
                    for av in out_avals]
    outs = sharded(*concat_in, *concat_zeros)
    o = np.asarray(outs[out_names.index("out")]).reshape(8, N, IN_DIM)
    full = np.empty((B, N_SRC, IN_DIM), np.float32)
    for c in range(8):
        bb, half = c // 2, c % 2
        full[bb, N * half:N * (half + 1)] = o[c]
    return full


# revision 20
# speedup vs baseline: 8.0182x; 8.0182x over previous
"""Bipartite graph attention layer on 8 Trainium2 NeuronCores.

Sharding: data-parallel over (batch b, n_src half). Core c handles
b = c // 2, rows n0 = (c % 2) * 1024 .. +1024 of feat_src; params and
feat_dst[b] replicated per b-pair.

Math (per batch b, head h):
  h_src = feat_src @ W[h]; h_dst = feat_dst @ W[h]
  s[n] = tanh(h_src[n]) . w_src[h];  d[m] = tanh(h_dst[m]) . w_dst[h]
  E[m, n] = exp(leaky_relu(s[n] + d[m], 0.2))
  feat_out[n] = (sum_m E[m,n] h_dst[m]) / (sum_m E[m,n]) + b

Key identity used to avoid materializing logits:
  leaky(x) = 0.2 x + 0.8 relu(x)  =>
  E = exp(0.2 s) * exp(0.2 d) * max(exp(0.8 s) exp(0.8 d), 1)
The per-n factor exp(0.2 s) cancels in the softmax ratio, so the kernel
accumulates E' = E / exp(0.2 s), elementwise over [128 m, 1024 n] tiles:
  DVE/Pool chunks:  E' = max(u8[n] * v1[m], v2[m])   (tensor_scalar)
  ACT chunks:       E' = relu(u8[n] * v1[m] - v2[m]) (activation Relu,
                    scale/bias per partition) + a per-head base term
                    sum_{m in ACT chunks} v2[m] rhs[m,:] folded into the
                    PSUM accumulation via a ones-row matmul
with u8 = exp(0.8 s), v1 = exp(d), v2 = exp(0.2 d). sum_m comes from an
extra ones-column appended to the matmul rhs. All fat f32 matmuls are
issued as float32r (1 PE cycle/row at >=256 moving cols vs 4 for f32).
Attention accumulates into two [128, 4*66] PSUM banks per head so the
softmax reciprocal batches 4 columns per instruction, and the
normalize+bias epilogue is a single scalar_tensor_tensor per n-tile.
sigmoid(g) is computed as 0.5 tanh(0.5 g) + 0.5 and elu via
relu(y) + exp(min(y, 0)) - 1 (stt-fused); Copy/Exp/Relu/Tanh share one
ACT table set so no table reloads occur.
"""

import sys

sys.path.insert(0, "/opt/trn_rl_repo")

import numpy as np

B, N_SRC, N_DST, IN_DIM, OUT_DIM, H = 4, 2048, 2048, 256, 64, 4
N = N_SRC // 2        # n_src rows per core
M = N_DST             # dst rows per core
NT = N // 128         # 8 n-tiles per core
MC = M // 128         # 16 m-chunks
RW = 66               # rhs width: 64 h_dst cols + ones col + pad col

_CACHE = {}

USE_F32R = True       # float32r fat matmuls (else plain f32, 4 cy/row)
USE_POOL = True       # gpsimd for Ep chunks / final chains / copies

# Ep chunk engine assignment per head: 'd' = DVE max-form,
# 'a' = ACT relu-form (+ base correction), 'p' = Pool max-form.
EP_ASSIGN = "aa" + "pp" + "dddddddddddd"
assert len(EP_ASSIGN) == MC


def _build_program(loop_k=None):
    import concourse.bass as bass
    import concourse.tile as tile
    from concourse import mybir
    from concourse.masks import make_identity

    f32 = mybir.dt.float32
    bf16 = mybir.dt.bfloat16
    AF = mybir.ActivationFunctionType
    OP = mybir.AluOpType

    nc = bass.Bass()
    fsrc_d = nc.declare_dram_parameter("fsrc", [N, IN_DIM], f32, isOutput=False)
    fdst_d = nc.declare_dram_parameter("fdst", [M, IN_DIM], f32, isOutput=False)
    W_d = nc.declare_dram_parameter("W", [H, IN_DIM, OUT_DIM], f32, isOutput=False)
    b_d = nc.declare_dram_parameter("bias", [OUT_DIM], f32, isOutput=False)
    wsrc_d = nc.declare_dram_parameter("wsrc", [H, OUT_DIM], f32, isOutput=False)
    wdst_d = nc.declare_dram_parameter("wdst", [H, OUT_DIM], f32, isOutput=False)
    Hw_d = nc.declare_dram_parameter("Hw", [IN_DIM, IN_DIM], f32, isOutput=False)
    Hb_d = nc.declare_dram_parameter("Hb", [IN_DIM], f32, isOutput=False)
    out_d = nc.declare_dram_parameter("out", [N, IN_DIM], f32, isOutput=True)

    with tile.TileContext(nc) as tc:
        if loop_k is None:
            _emit(nc, tc, bass, mybir, make_identity, f32, bf16, AF, OP,
                  fsrc_d, fdst_d, W_d, b_d, wsrc_d, wdst_d, Hw_d, Hb_d, out_d)
        else:
            with tc.For_i(0, loop_k):
                _emit(nc, tc, bass, mybir, make_identity, f32, bf16, AF, OP,
                      fsrc_d, fdst_d, W_d, b_d, wsrc_d, wdst_d, Hw_d, Hb_d,
                      out_d)

    _split_sync_waits(nc, mybir)
    return nc


def _emit(nc, tc, bass, mybir, make_identity, f32, bf16, AF, OP,
          fsrc_d, fdst_d, W_d, b_d, wsrc_d, wdst_d, Hw_d, Hb_d, out_d):
    from contextlib import ExitStack

    f32r = mybir.dt.float32r if USE_F32R else f32
    ep_assign = EP_ASSIGN if USE_POOL else EP_ASSIGN.replace("p", "d")

    ctx = ExitStack()
    with ctx:
        const = ctx.enter_context(tc.tile_pool(name="const", bufs=1))
        head_p = ctx.enter_context(tc.tile_pool(name="head", bufs=2))
        ep_p = ctx.enter_context(tc.tile_pool(name="ep", bufs=4))
        fin_p = ctx.enter_context(tc.tile_pool(name="fin", bufs=2))
        ps_sm = ctx.enter_context(tc.tile_pool(name="ps_sm", bufs=2, space="PSUM"))
        ps_acc = ctx.enter_context(tc.tile_pool(name="ps_acc", bufs=2, space="PSUM"))

        # ---------------- loads (small params first; batched 4-tile DMAs;
        # dispatch on SP + Pool so the ACT sequencer stays free) ----------
        W_sb = const.tile([128, 2, H, OUT_DIM], f32)
        W_src_ap = W_d.rearrange("h (c p) o -> c p h o", p=128)
        for c in range(2):
            nc.sync.dma_start(W_sb[:, c, :, :], W_src_ap[c])
        Hw_sb = const.tile([128, 2, IN_DIM], f32)
        nc.scalar.dma_start(Hw_sb, Hw_d.rearrange("(r p) i -> p r i", p=128))
        Hb_row = const.tile([1, IN_DIM], f32)
        nc.sync.dma_start(Hb_row, Hb_d[None, :])
        b_full = const.tile([1, H, OUT_DIM], f32)
        for h in range(H):
            nc.scalar.dma_start(b_full[:, h, :], b_d[None, :])
        # wsrc_colT[64*hh + o, pair] = w_src[2*pair + hh, o]
        wsrc_colT = const.tile([128, 2], f32)
        nc.sync.dma_start(wsrc_colT,
                          wsrc_d.rearrange("(pair hh) o -> (hh o) pair", hh=2))
        wdst_rows = const.tile([1, H, OUT_DIM], f32)
        nc.scalar.dma_start(wdst_rows, wdst_d[None, :, :])

        fdst_sb = const.tile([128, MC, IN_DIM], f32)
        fdst_ap = fdst_d.rearrange("(g r p) i -> g p r i", p=128, r=4)
        for gq in range(4):
            (nc.sync if gq % 2 == 0 else nc.scalar).dma_start(
                fdst_sb[:, 4 * gq:4 * (gq + 1), :], fdst_ap[gq])
        fsrc_sb = const.tile([128, NT, IN_DIM], f32)
        fsrc_ap = fsrc_d.rearrange("(g r p) i -> g p r i", p=128, r=4)
        for gq in range(2):
            (nc.sync if gq % 2 == 0 else nc.scalar).dma_start(
                fsrc_sb[:, 4 * gq:4 * (gq + 1), :], fsrc_ap[gq])

        ident = const.tile([128, 128], f32)
        make_identity(nc, ident)
        # f32r copies of DMA-delivered matmul operands (early: h_dst and
        # h_src matmuls consume W_r)
        if USE_F32R:
            W_r = const.tile([128, 2, H, OUT_DIM], f32r)
            nc.vector.tensor_copy(W_r.rearrange("p c h o -> p (c h o)"),
                                  W_sb.rearrange("p c h o -> p (c h o)"))
            Hb_row_r = const.tile([1, IN_DIM], f32r)
            (nc.gpsimd if USE_POOL else nc.vector).tensor_copy(
                Hb_row_r, Hb_row)
        else:
            W_r, Hb_row_r = W_sb, Hb_row
        ones_col = const.tile([1, 128], f32)
        nc.vector.memset(ones_col, 1.0)
        if USE_F32R:
            ones_col_r = const.tile([1, 128], f32r)
            nc.vector.tensor_copy(ones_col_r, ones_col)
        else:
            ones_col_r = ones_col
        ones512_b = const.tile([1, 512], bf16)
        nc.vector.memset(ones512_b, 1.0)
        ones128 = const.tile([128, 128], f32)
        nc.vector.memset(ones128, 1.0)

        # psum -> sbuf copies alternate DVE / ACT (GPSIMD cannot read PSUM)
        copy_rr = [nc.vector.tensor_copy, nc.scalar.copy]

        def pe_transpose4(dsts, srcs, ceng):
            # batch up to 4 [128,128] transposes into one psum bank + 1 copy
            ps = ps_sm.tile([128, 512], f32, tag="sm")
            for k, src in enumerate(srcs):
                nc.tensor.transpose(ps[:, 128 * k:128 * (k + 1)], src, ident)
            for k, dst in enumerate(dsts):
                if len(dsts) == 1:
                    ceng(dst, ps[:, 0:128 * len(srcs)])
                else:
                    ceng(dst, ps[:, 128 * k:128 * (k + 1)])

        def bcast_row(dst, row_ap, width):
            # dst[128, width] sbuf <- row_ap[1, width] replicated to all rows
            ps = ps_sm.tile([128, 512], f32, tag="sm")
            nc.tensor.matmul(ps[:, 0:width], ones_col, row_ap,
                             start=True, stop=True)
            nc.vector.tensor_copy(dst, ps[:, 0:width])

        # ---------------- transposes / broadcasts ----------------
        # transposed operand tiles are float32r so the fat projection
        # matmuls run at 1 PE cycle/row; their producers (copies / ACT)
        # round to f32r as the BIR verifier requires.
        fdstT = [const.tile([128, M], f32r, tag=f"fdstT{c}", name=f"fdstT{c}")
                 for c in range(2)]
        for c in range(2):
            for t0 in range(0, MC, 4):
                pe_transpose4(
                    [fdstT[c][:, 128 * t0:128 * (t0 + 4)]],
                    [fdst_sb[:, t, 128 * c:128 * (c + 1)]
                     for t in range(t0, t0 + 4)],
                    copy_rr[(t0 // 4 + c) % 2])
        fsrcT = [const.tile([128, N], f32r, tag=f"fsrcT{c}", name=f"fsrcT{c}")
                 for c in range(2)]
        for c in range(2):
            for t0 in range(0, NT, 4):
                pe_transpose4(
                    [fsrcT[c][:, 128 * t0:128 * (t0 + 4)]],
                    [fsrc_sb[:, t, 128 * c:128 * (c + 1)]
                     for t in range(t0, t0 + 4)],
                    copy_rr[(t0 // 4 + c) % 2])
        HwT = [const.tile([128, IN_DIM], f32r, tag=f"HwT{c}", name=f"HwT{c}")
               for c in range(2)]
        for c in range(2):
            pe_transpose4(
                [HwT[c][:, 0:256]],
                [Hw_sb[:, t, 128 * c:128 * (c + 1)] for t in range(2)],
                copy_rr[c % 2])
        # replicated w_src: wsrc_rep[64*hh + o, pair, c] = w_src[2*pair+hh, o]
        wsrc_rep = const.tile([128, 2, 128], f32r)
        for pair in range(2):
            nc.vector.tensor_scalar(wsrc_rep[:, pair, :], ones128,
                                    wsrc_colT[:, pair:pair + 1], None, OP.mult)
        # w_dst broadcast rows (bf16 so the tw mul gets the 2x DVE mode)
        wdst_row = const.tile([128, H, OUT_DIM], bf16)
        bcast_row(wdst_row.rearrange("p h o -> p (h o)"),
                  wdst_rows.rearrange("p h o -> p (h o)"), H * OUT_DIM)
        b_full_b = const.tile([128, H * OUT_DIM], f32)
        bcast_row(b_full_b, b_full.rearrange("p h o -> p (h o)"), H * OUT_DIM)

        # ---------------- h_dst for all heads + a_dst ----------------
        rhs_all = const.tile([128, H, MC, RW], bf16)
        nc.vector.memset(rhs_all[:, :, :, 64:65], 1.0)
        nc.vector.memset(rhs_all[:, :, :, 65:66], 0.0)
        th_all = const.tile([128, H, MC, OUT_DIM], bf16)
        a_dst = const.tile([128, H, MC], f32)
        v1 = const.tile([128, H, MC], f32)
        v2 = const.tile([128, H, MC], f32)
        negv2 = const.tile([128, H, MC], f32)
        n_act = ep_assign.count("a")
        if n_act:
            v2b = const.tile([128, H, MC], bf16)

        for mp in range(MC // 2):
            hd = ps_sm.tile([128, 2, H * OUT_DIM], f32, tag="hd", bufs=2)
            for q in range(2):
                mc = 2 * mp + q
                for c in range(2):
                    nc.tensor.matmul(
                        hd[:, q, :],
                        fdstT[c][:, 128 * mc:128 * (mc + 1)],
                        W_r[:, c, :, :].rearrange("p h o -> p (h o)"),
                        start=(c == 0), stop=(c == 1))
            # tanh straight from PSUM (ACT); bf16 rhs copy on DVE
            hd4 = hd.rearrange("p q (h o) -> p q h o", h=H)
            nc.scalar.activation(
                th_all[:, :, 2 * mp:2 * mp + 2, :].rearrange(
                    "p h q o -> p q h o"),
                hd4, AF.Tanh)
            nc.vector.tensor_copy(
                rhs_all[:, :, 2 * mp:2 * mp + 2, 0:OUT_DIM].rearrange(
                    "p h q o -> p q h o"),
                hd4)

        # ---------------- heads: attention (h_dst^T stationary, Ep moving;
        # 32 wide matmuls per head instead of 128 Ldweights+Matmult pairs) --
        feat_pre = const.tile([128, NT, H * OUT_DIM], bf16)
        for pair in range(2):
            # h_srcT for head pair: psum [128 (2h, o), 512] x2 blocks
            th_srcT = head_p.tile([128, N], f32r, tag="thsrc")
            for nb in range(2):
                hs = ps_sm.tile([128, 512], f32, tag="sm")
                for c in range(2):
                    nc.tensor.matmul(
                        hs,
                        W_r[:, c, 2 * pair:2 * pair + 2, :].rearrange(
                            "p h o -> p (h o)"),
                        fsrcT[c][:, 512 * nb:512 * (nb + 1)],
                        start=(c == 0), stop=(c == 1))
                nc.scalar.activation(th_srcT[:, 512 * nb:512 * (nb + 1)], hs,
                                     AF.Tanh)
            for hh in range(2):
                h = 2 * pair + hh
                # just-in-time per-head a_dst -> v1/v2 (keeps each engine's
                # queue position close to this head's Ep production)
                tw = head_p.tile([128, MC, OUT_DIM], bf16, tag="tw")
                nc.vector.tensor_mul(
                    tw, th_all[:, h, :, :],
                    wdst_row[:, h:h + 1, :].broadcast_to([128, MC, OUT_DIM]))
                nc.vector.tensor_reduce(a_dst[:, h, :], tw,
                                        mybir.AxisListType.X, OP.add)
                nc.scalar.activation(v1[:, h, :], a_dst[:, h, :], AF.Exp)
                nc.scalar.activation(v2[:, h, :], a_dst[:, h, :], AF.Exp,
                                     scale=0.2)
                if n_act:
                    nc.vector.tensor_scalar(negv2[:, h, :], v2[:, h, :],
                                            -1.0, None, OP.mult)
                    nc.vector.tensor_copy(v2b[:, h, :], v2[:, h, :])
                u8 = head_p.tile([128, N], bf16, tag="u8")
                for nb in range(2):
                    sb = ps_sm.tile([128, 512], f32, tag="sm")
                    nc.tensor.matmul(
                        sb, wsrc_rep[64 * hh:64 * (hh + 1), pair, :],
                        th_srcT[64 * hh:64 * (hh + 1),
                                512 * nb:512 * (nb + 1)],
                        start=True, stop=True)
                    nc.scalar.activation(u8[:, 512 * nb:512 * (nb + 1)], sb,
                                         AF.Exp, scale=0.8)
                # per-head base term over ACT-assigned chunks
                if n_act:
                    bps = ps_sm.tile([128, 512], f32, tag="hd", bufs=2,
                                     name=f"bps{h}")
                    first = True
                    for mc in range(MC):
                        if ep_assign[mc] != "a":
                            continue
                        nc.tensor.matmul(
                            bps[0:1, 0:RW], v2b[:, h, mc:mc + 1],
                            rhs_all[:, h, mc, :],
                            start=first, stop=(mc == MC - 1 or
                                               ep_assign[mc + 1:].count("a") == 0))
                        first = False
                    base_row = head_p.tile([1, RW], bf16, tag="base")
                    nc.vector.tensor_copy(base_row, bps[0:1, 0:RW])
                # produce all 16 Ep tiles (slow engines emitted first),
                # then two accB sweeps consume each tile twice
                eps = []
                for mc in range(MC):
                    e = ep_assign[mc]
                    Ep = ep_p.tile([128, N], bf16, tag="Ep", bufs=MC + 2,
                                   name=f"Ep{h}_{mc}")
                    if e == "a":
                        nc.scalar.activation(Ep, u8, AF.Relu,
                                             bias=negv2[:, h, mc:mc + 1],
                                             scale=v1[:, h, mc:mc + 1])
                    else:
                        eng = nc.vector if e == "d" else nc.gpsimd
                        eng.tensor_scalar(Ep, u8, v1[:, h, mc:mc + 1],
                                          v2[:, h, mc:mc + 1], OP.mult, OP.max)
                    eps.append(Ep)
                # accB[66, 512]: rows = 64 h_dst cols + denom + pad, cols = n.
                # Stationary rhs_all chunk, moving Ep half-tile.
                for nb in range(2):
                    accB = ps_acc.tile([66, 512], f32, tag="accB", bufs=2,
                                       name=f"accB{h}_{nb}")
                    if n_act:
                        nc.tensor.matmul(accB, base_row[:, 0:RW][0:1, 0:66],
                                         ones512_b, start=True, stop=False)
                    for mc in range(MC):
                        nc.tensor.matmul(
                            accB, rhs_all[:, h, mc, :],
                            eps[mc][:, 512 * nb:512 * (nb + 1)],
                            start=(mc == 0 and not n_act),
                            stop=(mc == MC - 1))
                    # drain: psum -> sbuf, transpose back, batched
                    # reciprocal, fused normalize+bias epilogue
                    acc_sb = head_p.tile([66, 512], f32, tag="accsb")
                    (nc.vector.tensor_copy if nb == 0 else nc.scalar.copy)(
                        acc_sb, accB)
                    accT = ps_acc.tile([128, 512], f32, tag="accT", bufs=2,
                                       name=f"accT{h}_{nb}")
                    for k in range(4):
                        nc.tensor.transpose(accT[:, 66 * k:66 * k + 66],
                                            acc_sb[:, 128 * k:128 * (k + 1)],
                                            ident[0:66, 0:66])
                    accT4 = accT[:, 0:264].rearrange("p (a b) -> p a b", a=4)
                    rec = ep_p.tile([128, 4], f32, tag="rec", bufs=4)
                    nc.vector.reciprocal(
                        rec, accT4[:, :, 64:65].rearrange("p a b -> p (a b)"))
                    for k in range(4):
                        ns = 4 * nb + k
                        nc.vector.scalar_tensor_tensor(
                            feat_pre[:, ns, 64 * h:64 * (h + 1)],
                            accT4[:, k, 0:64], rec[:, k:k + 1],
                            b_full_b[:, 64 * h:64 * (h + 1)],
                            OP.mult, OP.add)

        # ---------------- gate matmuls (sigmoid folded into final) -------
        # emitted after attention: the PE work slots into head-drain gaps
        fsrc_b = const.tile([128, NT, IN_DIM], bf16)
        tg_all = const.tile([128, NT, IN_DIM], bf16)
        for t in range(NT):
            g = ps_sm.tile([128, 512], f32, tag="hd", bufs=2,
                           name=f"g{t}")[:, 0:IN_DIM]
            for c in range(2):
                nc.tensor.matmul(g, fsrcT[c][:, 128 * t:128 * (t + 1)],
                                 HwT[c], start=(c == 0), stop=False)
            nc.tensor.matmul(g, ones_col_r, Hb_row_r,
                             start=False, stop=True)
            # tg = tanh(0.5 g); sigma(g) = 0.5 tg + 0.5 is folded into the
            # final combine: out = 0.5 (tg + 1) (z-1 - fsrc) + fsrc
            nc.scalar.activation(tg_all[:, t, :], g, AF.Tanh, scale=0.5)
            (nc.vector if (t % 2 or not USE_POOL) else nc.gpsimd).tensor_copy(
                fsrc_b[:, t, :], fsrc_sb[:, t, :])

        # ---------------- elu + gate + combine ----------------
        # y = feat_pre (bias already added); z-1 = elu(y);
        # out = 0.5 (tg+1) (z-1-fsrc) + fsrc
        for t in range(NT):
            dve = (not USE_POOL) or t in (0, 1, 3, 4, 6, 7)
            v = nc.vector if dve else nc.gpsimd
            y = feat_pre[:, t, :]
            mn = fin_p.tile([128, IN_DIM], bf16, tag="mn")
            v.tensor_scalar(mn, y, 0.0, None, OP.min)
            e = fin_p.tile([128, IN_DIM], bf16, tag="e")
            nc.scalar.activation(e, mn, AF.Exp)
            d = fin_p.tile([128, IN_DIM], bf16, tag="d")
            q = fin_p.tile([128, IN_DIM], bf16, tag="q")
            if dve:
                # z-1 = (max(y,0) + e) - 1; d = z-1 - fsrc  (stt-fused)
                z1 = fin_p.tile([128, IN_DIM], bf16, tag="z1")
                v.scalar_tensor_tensor(z1, y, 0.0, e, OP.max, OP.add)
                v.scalar_tensor_tensor(d, z1, -1.0, fsrc_b[:, t, :],
                                       OP.add, OP.subtract)
                v.scalar_tensor_tensor(q, tg_all[:, t, :], 1.0, d,
                                       OP.add, OP.mult)
            else:
                # Pool has no scalar_tensor_tensor
                rr = fin_p.tile([128, IN_DIM], bf16, tag="rr")
                v.tensor_scalar(rr, y, 0.0, -1.0, OP.max, OP.add)
                z1 = fin_p.tile([128, IN_DIM], bf16, tag="z1")
                v.tensor_add(z1, rr, e)
                v.tensor_sub(d, z1, fsrc_b[:, t, :])
                tp = fin_p.tile([128, IN_DIM], bf16, tag="tp")
                v.tensor_scalar(tp, tg_all[:, t, :], 1.0, None, OP.add)
                v.tensor_mul(q, d, tp)
            o = fin_p.tile([128, IN_DIM], f32, tag="o")
            (nc.vector.scalar_tensor_tensor if dve
             else nc.vector.scalar_tensor_tensor)(
                o, q, 0.5, fsrc_sb[:, t, :], OP.mult, OP.add)
            (nc.sync if t % 2 == 0 else nc.scalar).dma_start(
                out_d[128 * t:128 * (t + 1), :], o)


def _split_sync_waits(nc, mybir, max_waits=1, drain_max_waits=0):
    """Walrus for cayman here accepts at most one sem-wait per
    instruction (and none on Drain): move overflow waits onto preceding
    same-engine NOPs."""
    n_split = 0
    for f in nc.m.functions:
        for bb in f.blocks:
            il = bb.instructions
            i = 0
            while i < len(il):
                ins = il[i]
                si = ins.sync_info
                limit = (drain_max_waits
                         if type(ins).__name__ == "InstDrain" else max_waits)
                if si is not None and len(si.on_wait) > limit:
                    waits = list(si.on_wait)
                    keep = waits[-limit:] if limit > 0 else []
                    overflow = waits[:len(waits) - limit]
                    chunks = [overflow[j:j + max_waits]
                              for j in range(0, len(overflow), max_waits)]
                    pos = i
                    for chunk in chunks:
                        nop = mybir.InstNoOp(
                            name=f"I-waitsplit-{n_split}",
                            engine=ins.engine,
                            sync_info=mybir.SyncInfo(on_wait=chunk, on_update=[]),
                        )
                        n_split += 1
                        il.insert(pos, nop)
                        pos += 1
                        i += 1
                    ins.sync_info = mybir.SyncInfo(
                        on_wait=keep, on_update=list(si.on_update))
                i += 1
    return n_split


def _get_runner():
    if "runner" in _CACHE:
        return _CACHE["runner"]
    import jax
    from jax.sharding import Mesh, PartitionSpec
    from jax.experimental.shard_map import shard_map
    import concourse.mybir as mybir
    from concourse.bass2jax import (_bass_exec_p, install_neuronx_cc_hook,
                                    partition_id_tensor)

    nc = _build_program()
    install_neuronx_cc_hook()
    n_cores = 8

    in_names, out_names, out_avals = [], [], []
    for alloc in nc.m.functions[0].allocations:
        if not isinstance(alloc, mybir.MemoryLocationSet):
            continue
        name = alloc.memorylocations[0].name
        if alloc.kind == "ExternalInput":
            if (nc.partition_id_tensor is not None
                    and name == nc.partition_id_tensor.name):
                continue
            in_names.append(name)
        elif alloc.kind == "ExternalOutput":
            out_names.append(name)
            out_avals.append(jax.core.ShapedArray(
                tuple(alloc.tensor_shape), mybir.dt.np(alloc.dtype)))
    n_params = len(in_names)
    in_names_all = list(in_names) + list(out_names)
    if nc.partition_id_tensor is not None:
        in_names_all.append(nc.partition_id_tensor.name)

    def _body(*args):
        operands = list(args)
        if nc.partition_id_tensor is not None:
            operands.append(partition_id_tensor())
        return tuple(_bass_exec_p.bind(
            *operands,
            out_avals=tuple(out_avals),
            in_names=tuple(in_names_all),
            out_names=tuple(out_names),
            lowering_input_output_aliases=(),
            sim_require_finite=True,
            sim_require_nnan=True,
            nc=nc,
        ))

    devices = jax.devices()[:n_cores]
    mesh = Mesh(np.asarray(devices), ("core",))
    n_outs = len(out_names)
    sharded = jax.jit(
        shard_map(_body, mesh=mesh,
                  in_specs=(PartitionSpec("core"),) * (n_params + n_outs),
                  out_specs=(PartitionSpec("core"),) * n_outs,
                  check_rep=False),
        keep_unused=True,
    )
    runner = (sharded, in_names, out_names, out_avals)
    _CACHE["runner"] = runner
    return runner


def _shard_inputs(feat_src, feat_dst, W, b, w_src, w_dst, H_w, H_b):
    per_core = []
    for c in range(8):
        bb, half = c // 2, c % 2
        per_core.append({
            "fsrc": np.ascontiguousarray(feat_src[bb, N * half:N * (half + 1)]),
            "fdst": np.ascontiguousarray(feat_dst[bb]),
            "W": W, "bias": b, "wsrc": w_src, "wdst": w_dst,
            "Hw": H_w, "Hb": H_b,
        })
    return per_core


def kernel(feat_src, feat_dst, W, b, w_src, w_dst, H_w, H_b):
    feat_src = np.asarray(feat_src, np.float32)
    feat_dst = np.asarray(feat_dst, np.float32)
    args = [np.asarray(a, np.float32) for a in (W, b, w_src, w_dst, H_w, H_b)]
    sharded, in_names, out_names, out_avals = _get_runner()
    per_core = _shard_inputs(feat_src, feat_dst, *args)
    concat_in = [np.concatenate([per_core[c][nm] for c in range(8)], axis=0)
                 for nm in in_names]
    concat_zeros = [np.zeros((8 * av.shape[0], *av.shape[1:]), av.dtype)
                    for av in out_avals]
    outs = sharded(*concat_in, *concat_zeros)
    o = np.asarray(outs[out_names.index("out")]).reshape(8, N, IN_DIM)
    full = np.empty((B, N_SRC, IN_DIM), np.float32)
    for c in range(8):
        bb, half = c // 2, c % 2
        full[bb, N * half:N * (half + 1)] = o[c]
    return full
